# revision 34
# baseline (speedup 1.0000x reference)
"""Trainium2 Bass kernel for the MoE Conditional Neural Process problem.

Contract: kernel(**inputs) takes the FULL unsharded inputs (as produced by
reference.setup_inputs()) and returns the FULL output tuple
(mu_c, lv_c, mu_t, lv_t, y_mean, y_std, alpha_post, alpha_prior).

Sharding: data-parallel over batch B=8 across the 8 NeuronCores (one batch
element per core). Weights are replicated. All distribution / gather is done
host-side in this file.

Device kernel design (per core, batch element b):
  - activations kept feature-major: [features on partitions, points on free]
  - all matmuls in float32r (tf32-class precision, full PE rate)
  - encoder: per expert k, 3-layer MLP on 512 points (x2 sets), mean-pool
    fused into the last relu via accum_out, then mu/lv heads (N=1 matmuls)
  - z = mu_t + eps * exp(0.5 lv_t) on-chip
  - decoder/gates: input concat(x, z_k) -> the z part is constant per k, so
    W0_z @ z_k is folded into a per-expert bias; the x projection is computed
    once (k-independent) and reused for all 8 experts
  - gate logits computed row-major ([128 rows, 1] matmuls with the hidden
    activations as the stationary operand), softmax over K on-chip
"""

import numpy as np

B, NC, NT, T = 8, 512, 512, 1024
DX, DY, L, K = 2, 3, 128, 8
H, NH = 512, 2
DH, NDH = 512, 3
HG, NG = 256, 1

_CACHE = {}


def _build():
    import concourse.mybir as mybir
    import concourse.tile as tile
    from concourse import bacc

    f32 = mybir.dt.float32
    f32r = mybir.dt.float32r
    AF = mybir.ActivationFunctionType
    ALU = mybir.AluOpType
    AX = mybir.AxisListType

    nc = bacc.Bacc(trn_type="TRN2", target_bir_lowering=False, debug=False)

    # ---------------- DRAM I/O ----------------
    d_enc_in = nc.dram_tensor("enc_in", [5, 2, 512], f32r, kind="ExternalInput")
    d_xyT = nc.dram_tensor("xyT", [5, T], f32r, kind="ExternalInput")
    d_epsT = nc.dram_tensor("epsT", [128, K], f32, kind="ExternalInput")

    d_ew0 = nc.dram_tensor("ew0", [K, 5, H], f32r, kind="ExternalInput")
    d_ewh = nc.dram_tensor("ewh", [K, 128, NH, 4, H], f32r, kind="ExternalInput")
    d_ewml = nc.dram_tensor("ewml", [K, 128, 4, 2 * L], f32r, kind="ExternalInput")
    d_ebias = nc.dram_tensor("ebias", [128, K, 3, 4], f32, kind="ExternalInput")
    d_emlb = nc.dram_tensor("emlb", [128, K, 2], f32, kind="ExternalInput")

    d_dw0x = nc.dram_tensor("dw0x", [2, DH], f32r, kind="ExternalInput")
    d_dw0z = nc.dram_tensor("dw0z", [128, DH], f32r, kind="ExternalInput")
    d_dwh = nc.dram_tensor("dwh", [128, NDH, 4, DH], f32r, kind="ExternalInput")
    d_dwo = nc.dram_tensor("dwo", [128, 4, 35], f32r, kind="ExternalInput")
    d_dbias = nc.dram_tensor("dbias", [128, 4, 4], f32, kind="ExternalInput")
    d_dbo = nc.dram_tensor("dbo", [35, 1], f32, kind="ExternalInput")

    d_pw0i = nc.dram_tensor("pw0i", [5, HG], f32r, kind="ExternalInput")
    d_pw0z = nc.dram_tensor("pw0z", [128, HG], f32r, kind="ExternalInput")
    d_pwh = nc.dram_tensor("pwh", [128, 2, HG], f32r, kind="ExternalInput")
    d_pwo = nc.dram_tensor("pwo", [128, 2, 1], f32r, kind="ExternalInput")
    d_pb = nc.dram_tensor("pb", [128, 2, 2], f32, kind="ExternalInput")

    d_qw0i = nc.dram_tensor("qw0i", [2, HG], f32r, kind="ExternalInput")
    d_qw0z = nc.dram_tensor("qw0z", [128, HG], f32r, kind="ExternalInput")
    d_qwh = nc.dram_tensor("qwh", [128, 2, HG], f32r, kind="ExternalInput")
    d_qwo = nc.dram_tensor("qwo", [128, 2, 1], f32r, kind="ExternalInput")
    d_qb = nc.dram_tensor("qb", [128, 2, 2], f32, kind="ExternalInput")

    d_lg = nc.dram_tensor("lg_scratch", [2, K, T], f32, kind="Internal")
    d_enc_out = nc.dram_tensor("enc_out", [128, 2, 2, K], f32, kind="ExternalOutput")
    d_dec_out = nc.dram_tensor("dec_out", [K, 6, T], f32, kind="ExternalOutput")
    d_ap_out = nc.dram_tensor("alpha_post", [T, K], f32, kind="ExternalOutput")
    d_aq_out = nc.dram_tensor("alpha_prior", [T, K], f32, kind="ExternalOutput")

    alt = [0]

    with tile.TileContext(nc) as tc:
        import contextlib

        with contextlib.ExitStack() as ctx:
            consts = ctx.enter_context(tc.tile_pool(name="consts", bufs=1))
            encw = ctx.enter_context(tc.tile_pool(name="encw", bufs=2))
            acts = ctx.enter_context(tc.tile_pool(name="acts", bufs=2))
            acts3 = ctx.enter_context(tc.tile_pool(name="acts3", bufs=2))
            # g0 lives across expert iterations: 4 in flight + 4 draining
            g0pool = ctx.enter_context(tc.tile_pool(name="g0pool", bufs=5))
            ghpool = ctx.enter_context(tc.tile_pool(name="ghpool", bufs=4))
            xproj = ctx.enter_context(tc.tile_pool(name="xproj", bufs=1))
            persist = ctx.enter_context(tc.tile_pool(name="persist", bufs=1))
            small = ctx.enter_context(tc.tile_pool(name="small", bufs=2))
            smx = ctx.enter_context(tc.tile_pool(name="smx", bufs=8))
            outs = ctx.enter_context(tc.tile_pool(name="outs", bufs=2))
            ps_h = ctx.enter_context(tc.tile_pool(name="ps_h", bufs=3, space="PSUM"))
            ps_s = ctx.enter_context(tc.tile_pool(name="ps_s", bufs=2, space="PSUM"))
            ps_l = ctx.enter_context(tc.tile_pool(name="ps_l", bufs=1, space="PSUM"))
            ps_o = ctx.enter_context(tc.tile_pool(name="ps_o", bufs=2, space="PSUM"))

            zcol = None

            def relu_store(out, in_, bias, accum_out=None, sbuf_src=False):
                """relu(in_ + bias) -> out, alternating ACT / DVE.

                NB: tensor_scalar with scalar1=AP and scalar2=immediate
                silently drops op1 on this toolchain; scalar2 must be an AP.
                """
                alt[0] ^= 1
                if alt[0] or accum_out is not None:
                    # DVE tensor_scalar with accum_out corrupts both outputs
                    # on this toolchain -- keep accumulating relus on ACT
                    nc.scalar.activation(
                        out=out, in_=in_, func=AF.Relu, bias=bias, accum_out=accum_out
                    )
                else:
                    nc.vector.tensor_scalar(
                        out, in_, bias, zcol[:, 0:1], ALU.add, ALU.max
                    )

            def mm32(ps, lhsT, rhs, start, stop):
                # N=1 matmuls: fp32r rejects free-size-1 moving operands in
                # codegen; run them as plain fp32 (cost is negligible at N=1)
                nc.tensor.matmul(
                    ps, lhsT.bitcast(f32), rhs.bitcast(f32), start=start, stop=stop
                )

            def load_enc(k):
                t0 = encw.tile([5, H], f32r, tag="ew0")
                nc.sync.dma_start(t0[:], d_ew0[k])
                t1 = encw.tile([128, NH, 4, H], f32r, tag="ewh")
                nc.sync.dma_start(t1[:], d_ewh[k])
                t2 = encw.tile([128, 4, 2 * L], f32r, tag="ewml")
                nc.sync.dma_start(t2[:], d_ewml[k])
                return t0, t1, t2

            # ---------------- constant loads ----------------
            # order matters: the Sync DMA queue drains in order, so small
            # tensors needed by the first compute (xproj, encoder L0) go
            # first, then the k=0 encoder weights (2.4 MB), then the rest
            zcol = consts.tile([128, 1], f32)
            nc.vector.memset(zcol[:], 0.0)
            xyT = consts.tile([5, T], f32r)
            nc.sync.dma_start(xyT[:], d_xyT[:])
            xT = xyT[0:2, :]
            dw0x = consts.tile([2, DH], f32r)
            nc.sync.dma_start(dw0x[:], d_dw0x[:])
            pw0i = consts.tile([5, HG], f32r)
            nc.sync.dma_start(pw0i[:], d_pw0i[:])
            qw0i = consts.tile([2, HG], f32r)
            nc.sync.dma_start(qw0i[:], d_qw0i[:])
            enc_in = consts.tile([5, 2, 512], f32r)
            nc.sync.dma_start(enc_in[:], d_enc_in[:])
            ebias = consts.tile([128, K, 3, 4], f32)
            nc.sync.dma_start(ebias[:], d_ebias[:])

            pending_encw = load_enc(0)

            epsT = consts.tile([128, K], f32)
            nc.sync.dma_start(epsT[:], d_epsT[:])
            emlb = consts.tile([128, K, 2], f32)
            nc.sync.dma_start(emlb[:], d_emlb[:])
            dbias = consts.tile([128, 4, 4], f32)
            nc.sync.dma_start(dbias[:], d_dbias[:])
            dbo = consts.tile([35, 1], f32)
            nc.sync.dma_start(dbo[:], d_dbo[:])
            pb = consts.tile([128, 2, 2], f32)
            nc.sync.dma_start(pb[:], d_pb[:])
            qb = consts.tile([128, 2, 2], f32)
            nc.sync.dma_start(qb[:], d_qb[:])

            # persistent state
            zT = persist.tile([128, K], f32r)
            enc_sb = persist.tile([128, 2, 2, K], f32)
            asb_p = persist.tile([128, 8, K], f32)
            asb_q = persist.tile([128, 8, K], f32)

            # ---------------- x projections (k-independent) ----------------
            xpd = xproj.tile([128, 4, T], f32)
            xpp = xproj.tile([128, 2, T], f32)
            xpq = xproj.tile([128, 2, T], f32)
            for wsb, xin, n_o, xp in (
                (dw0x, xT, 4, xpd),
                (pw0i, xyT, 2, xpp),
                (qw0i, xT, 2, xpq),
            ):
                for c in range(2):
                    for o in range(n_o):
                        ps = ps_h.tile([128, 512], f32, tag="ph")
                        nc.tensor.matmul(
                            ps[:],
                            wsb[:, o * 128 : (o + 1) * 128],
                            xin[:, c * 512 : (c + 1) * 512],
                            start=True,
                            stop=True,
                        )
                        nc.vector.tensor_copy(xp[:, o, c * 512 : (c + 1) * 512], ps[:])

            # ---------------- encoders ----------------
            for k in range(K):
                ew0_k, ewh_k, ewml_k = pending_encw
                if k + 1 < K:
                    pending_encw = load_enc(k + 1)
                rr_all = small.tile([128, 4, 2], f32r, tag="rr")

                for s in range(2):
                    h = acts.tile([128, 4, 512], f32r, tag="ench")
                    for o in range(4):
                        ps = ps_h.tile([128, 512], f32, tag="ph")
                        nc.tensor.matmul(
                            ps[:],
                            ew0_k[:, o * 128 : (o + 1) * 128],
                            enc_in[:, s, :],
                            start=True,
                            stop=True,
                        )
                        relu_store(h[:, o, :], ps[:], ebias[:, k, 0, o : o + 1])
                    rsum = small.tile([128, 4], f32, tag="rsum")
                    for l in (1, 2):
                        hn = acts.tile([128, 4, 512], f32r, tag="ench")
                        for o in range(4):
                            ps = ps_h.tile([128, 512], f32, tag="ph")
                            for i in range(4):
                                nc.tensor.matmul(
                                    ps[:],
                                    ewh_k[:, l - 1, i, o * 128 : (o + 1) * 128],
                                    h[:, i, :],
                                    start=(i == 0),
                                    stop=(i == 3),
                                )
                            relu_store(
                                hn[:, o, :],
                                ps[:],
                                ebias[:, k, l, o : o + 1],
                                accum_out=(rsum[:, o : o + 1] if l == 2 else None),
                            )
                        h = hn
                    nc.vector.tensor_copy(rr_all[:, :, s], rsum[:])
                # mu/lv heads for both sets at once (N=2); the 1/512 mean
                # factor is folded into the bias-add below
                ps_mu = ps_s.tile([128, 2], f32, tag="pss")
                for i in range(4):
                    nc.tensor.matmul(
                        ps_mu[:], ewml_k[:, i, 0:L], rr_all[:, i, :],
                        start=(i == 0), stop=(i == 3),
                    )
                ps_lv = ps_s.tile([128, 2], f32, tag="pss")
                for i in range(4):
                    nc.tensor.matmul(
                        ps_lv[:], ewml_k[:, i, L : 2 * L], rr_all[:, i, :],
                        start=(i == 0), stop=(i == 3),
                    )
                for s in range(2):
                    nc.vector.scalar_tensor_tensor(
                        out=enc_sb[:, s, 0, k : k + 1],
                        in0=ps_mu[:, s : s + 1],
                        scalar=1.0 / 512.0,
                        in1=emlb[:, k, 0:1],
                        op0=ALU.mult,
                        op1=ALU.add,
                    )
                    nc.vector.scalar_tensor_tensor(
                        out=enc_sb[:, s, 1, k : k + 1],
                        in0=ps_lv[:, s : s + 1],
                        scalar=1.0 / 512.0,
                        in1=emlb[:, k, 1:2],
                        op0=ALU.mult,
                        op1=ALU.add,
                    )
                ze = small.tile([128, 1], f32, tag="ze")
                nc.scalar.activation(
                    out=ze[:],
                    in_=enc_sb[:, 1, 1, k : k + 1],
                    func=AF.Exp,
                    scale=0.5,
                )
                zm = small.tile([128, 1], f32, tag="zm")
                nc.vector.tensor_mul(zm[:], ze[:], epsT[:, k : k + 1])
                nc.vector.tensor_add(
                    zT[:, k : k + 1], zm[:], enc_sb[:, 1, 0, k : k + 1]
                )
            nc.sync.dma_start(d_enc_out[:], enc_sb[:])

            # phase-D weights: loaded after the encoder weights so the
            # startup DMA queue serves the encoder first (these are not
            # needed until the encoder phase is done)
            dw0z = consts.tile([128, DH], f32r)
            nc.sync.dma_start(dw0z[:], d_dw0z[:])
            dwh = consts.tile([128, NDH, 4, DH], f32r)
            nc.sync.dma_start(dwh[:], d_dwh[:])
            dwo = consts.tile([128, 4, 35], f32r)
            nc.sync.dma_start(dwo[:], d_dwo[:])
            pw0z = consts.tile([128, HG], f32r)
            nc.sync.dma_start(pw0z[:], d_pw0z[:])
            pwh = consts.tile([128, 2, HG], f32r)
            nc.sync.dma_start(pwh[:], d_pwh[:])
            pwo = consts.tile([128, 2, 1], f32r)
            nc.sync.dma_start(pwo[:], d_pwo[:])
            qw0z = consts.tile([128, HG], f32r)
            nc.sync.dma_start(qw0z[:], d_qw0z[:])
            qwh = consts.tile([128, 2, HG], f32r)
            nc.sync.dma_start(qwh[:], d_qwh[:])
            qwo = consts.tile([128, 2, 1], f32r)
            nc.sync.dma_start(qwo[:], d_qwo[:])
            # ---------------- z-dependent biases, all experts at once ----
            # zball slots: 0..3 decoder o-tiles, 4..5 post gate, 6..7 prior
            zball = persist.tile([128, 8, K], f32)
            zb_specs = (
                [(dw0z, o, dbias[:, 0, o : o + 1], o) for o in range(4)]
                + [(pw0z, o, pb[:, 0, o : o + 1], 4 + o) for o in range(2)]
                + [(qw0z, o, qb[:, 0, o : o + 1], 6 + o) for o in range(2)]
            )
            for zw, o, bias_ap, slot in zb_specs:
                ps = ps_s.tile([128, K], f32, tag="pss")
                nc.tensor.matmul(
                    ps[:], zw[:, o * 128 : (o + 1) * 128], zT[:], start=True, stop=True
                )
                nc.vector.tensor_scalar(
                    zball[:, slot, :], ps[:], bias_ap, zcol[:, 0:1], ALU.add, ALU.add
                )

            # ---------------- per-expert gates + decoder ----------------
            def emit_g0(k):
                # gate L0 relus for expert k; emitted one expert ahead so the
                # ACT/DVE engines produce them while the PE runs the previous
                # expert's decoder (kills the PE stall at expert boundaries)
                res = []
                for xp, zslice in (
                    (xpp, zball[:, 4:6, k : k + 1]),
                    (xpq, zball[:, 6:8, k : k + 1]),
                ):
                    per_c = []
                    for c in range(2):
                        g0 = g0pool.tile([128, 2, 512], f32r, tag="g0")
                        for o in range(2):
                            relu_store(
                                g0[:, o, :],
                                xp[:, o, c * 512 : (c + 1) * 512],
                                zslice[:, o, :],
                                sbuf_src=True,
                            )
                        per_c.append(g0)
                    res.append(per_c)
                return res

            def gate_stage1(k, gi):
                # gate hidden layer (PE + relu); g0 was produced during the
                # previous expert's decoder
                wh, gb = ((pwh, pb), (qwh, qb))[gi]
                per_c = []
                for c in range(2):
                    g0 = g0_cur[gi][c]
                    g1 = ghpool.tile([128, 2, 512], f32r, tag="gh")
                    for o in range(2):
                        ps = ps_h.tile([128, 512], f32, tag="ph")
                        for i in range(2):
                            nc.tensor.matmul(
                                ps[:],
                                wh[:, i, o * 128 : (o + 1) * 128],
                                g0[:, i, :],
                                start=(i == 0),
                                stop=(i == 1),
                            )
                        relu_store(g1[:, o, :], ps[:], gb[:, 1, o : o + 1])
                    per_c.append(g1)
                return per_c

            def gate_stage2(k, gi, g1s):
                # logits (weight-stationary, M=1) + softmax-layout scatter
                wo = (pwo, qwo)[gi]
                asb = (asb_p, asb_q)[gi]
                for c in range(2):
                    psl = ps_l.tile([1, 512], f32, tag="psl")
                    for i in range(2):
                        nc.tensor.matmul(
                            psl[:],
                            wo[:, i, 0:1],
                            g1s[c][:, i, :],
                            start=(i == 0),
                            stop=(i == 1),
                        )
                    lgc = outs.tile([1, 512], f32, tag="lg")
                    nc.vector.tensor_copy(lgc[:], psl[:])
                    nc.sync.dma_start(d_lg[gi, k, c * 512 : (c + 1) * 512], lgc[:])
                nc.sync.dma_start(
                    asb[:, :, k],
                    d_lg[gi, k].rearrange("(c p) -> p c", p=128),
                )

            def emit_softmax():
                for asb, dout in ((asb_p, d_ap_out), (asb_q, d_aq_out)):
                    for r in range(8):
                        nm = smx.tile([128, 1], f32, tag="sm")
                        nc.vector.tensor_reduce(
                            out=nm[:], in_=asb[:, r, :], axis=AX.X, op=ALU.max
                        )
                        nc.scalar.mul(nm[:], nm[:], -1.0)
                        e = smx.tile([128, K], f32, tag="se")
                        nc.scalar.activation(
                            out=e[:], in_=asb[:, r, :], func=AF.Exp, bias=nm[:, 0:1]
                        )
                        ssum = smx.tile([128, 1], f32, tag="ss")
                        nc.vector.tensor_reduce(
                            out=ssum[:], in_=e[:], axis=AX.X, op=ALU.add
                        )
                        rec = smx.tile([128, 1], f32, tag="sr")
                        nc.vector.reciprocal_approx_fast(out=rec[:], in_=ssum[:])
                        a = smx.tile([128, K], f32, tag="sa")
                        nc.vector.tensor_scalar_mul(a[:], e[:], rec[:, 0:1])
                        nc.sync.dma_start(dout[r * 128 : (r + 1) * 128, :], a[:])

            # the gate pipeline for expert k is spread through the decoder of
            # expert k, stage by stage, so the PE stream never drains on the
            # gate latency chain (matmul -> relu -> matmul -> copy)
            g0_cur = emit_g0(0)
            g1_cur = [gate_stage1(0, 0), gate_stage1(0, 1)]
            for k in range(K):
                zb_d = zball[:, 0:4, k : k + 1]
                ymo = outs.tile([35, T], f32, tag="ymo")
                for c in range(2):
                    h = acts3.tile([128, 4, 512], f32r, tag="dech")
                    for o in range(4):
                        relu_store(
                            h[:, o, :],
                            xpd[:, o, c * 512 : (c + 1) * 512],
                            zb_d[:, o, :],
                            sbuf_src=True,
                        )
                    for l in range(NDH):
                        hn = acts3.tile([128, 4, 512], f32r, tag="dech")
                        for o in range(4):
                            ps = ps_h.tile([128, 512], f32, tag="ph")
                            for i in range(4):
                                nc.tensor.matmul(
                                    ps[:],
                                    dwh[:, l, i, o * 128 : (o + 1) * 128],
                                    h[:, i, :],
                                    start=(i == 0),
                                    stop=(i == 3),
                                )
                            relu_store(hn[:, o, :], ps[:], dbias[:, l + 1, o : o + 1])
                        h = hn
                        if c == 0 and l == 0:
                            gate_stage2(k, 0, g1_cur[0])
                        elif c == 0 and l == 1:
                            gate_stage2(k, 1, g1_cur[1])
                            if k == K - 1:
                                emit_softmax()
                        elif c == 0 and l == 2 and k + 1 < K:
                            g0_cur = emit_g0(k + 1)
                        elif c == 1 and l == 0 and k + 1 < K:
                            g1_cur[0] = gate_stage1(k + 1, 0)
                        elif c == 1 and l == 1 and k + 1 < K:
                            g1_cur[1] = gate_stage1(k + 1, 1)
                    pso = ps_o.tile([35, 512], f32, tag="po")
                    for i in range(4):
                        nc.tensor.matmul(
                            pso[:],
                            dwo[:, i, :],
                            h[:, i, :],
                            start=(i == 0),
                            stop=(i == 3),
                        )
                    # sigmoid(x+b) = 1/(1+exp(-(x+b)));  dbo holds -b in rows
                    # 0:3 and +b in rows 3:6 (prepared host-side)
                    et = small.tile([35, 512], f32, tag="eo")
                    nc.scalar.activation(
                        out=et[0:3, :],
                        in_=pso[0:3, :],
                        func=AF.Exp,
                        bias=dbo[0:3, 0:1],
                        scale=-1.0,
                    )
                    nc.scalar.activation(
                        out=et[32:35, :],
                        in_=pso[32:35, :],
                        func=AF.Exp,
                        bias=dbo[32:35, 0:1],
                    )
                    nc.vector.tensor_scalar_add(et[:], et[:], 1.0)
                    nc.vector.reciprocal_approx_fast(
                        out=ymo[0:3, c * 512 : (c + 1) * 512], in_=et[0:3, :]
                    )
                    # softplus(x+b) = ln(1 + exp(x+b))
                    nc.scalar.activation(
                        out=ymo[32:35, c * 512 : (c + 1) * 512],
                        in_=et[32:35, :],
                        func=AF.Ln,
                    )
                nc.sync.dma_start(d_dec_out[k, 0:3], ymo[0:3, :])
                nc.sync.dma_start(d_dec_out[k, 3:6], ymo[32:35, :])


    nc.compile()
    return nc


def _prep_shared(inp):
    """Host-side weight layout transforms (same for all cores)."""
    f = np.ascontiguousarray
    eW0, eb0, eWh, ebh = inp["eW0"], inp["eb0"], inp["eWh"], inp["ebh"]
    eWmu, ebmu, eWlv, eblv = inp["eWmu"], inp["ebmu"], inp["eWlv"], inp["eblv"]
    pW0, pb0, pWh, pbh, pWo = inp["pW0"], inp["pb0"], inp["pWh"], inp["pbh"], inp["pWo"]
    qW0, qb0, qWh, qbh, qWo = inp["qW0"], inp["qb0"], inp["qWh"], inp["qbh"], inp["qWo"]
    dW0, db0, dWh, dbh, dWo, dbo = (
        inp["dW0"], inp["db0"], inp["dWh"], inp["dbh"], inp["dWo"], inp["dbo"],
    )

    out = {}
    out["ew0"] = f(eW0)  # [K, 5, H]
    # ewh[k, p, l, i, o] = eWh[l, k, i*128+p, o]
    ewh = eWh.reshape(NH, K, 4, 128, H).transpose(1, 3, 0, 2, 4)
    out["ewh"] = f(ewh)
    # ewml[k, p, i, :128]=eWmu[k, i*128+p, :], [128:]=eWlv
    wmu = eWmu.reshape(K, 4, 128, L).transpose(0, 2, 1, 3)
    wlv = eWlv.reshape(K, 4, 128, L).transpose(0, 2, 1, 3)
    out["ewml"] = f(np.concatenate([wmu, wlv], axis=-1))  # [K,128,4,256]
    # ebias[p, k, l, o]: l=0 -> eb0[k, o*128+p]; l=1,2 -> ebh[l-1, k, o*128+p]
    eb_all = np.stack([eb0, ebh[0], ebh[1]], axis=1)  # [K, 3, H]
    out["ebias"] = f(eb_all.reshape(K, 3, 4, 128).transpose(3, 0, 1, 2))
    out["emlb"] = f(np.stack([ebmu, eblv], axis=-1).transpose(1, 0, 2))  # [128,K,2]

    out["dw0x"] = f(dW0[:2])  # [2, DH]
    out["dw0z"] = f(dW0[2:])  # [128, DH]
    out["dwh"] = f(dWh.reshape(NDH, 4, 128, DH).transpose(2, 0, 1, 3))  # [128,3,4,DH]
    dwo_t = dWo.reshape(4, 128, 6).transpose(1, 0, 2)  # [128,4,6]
    dwo_pad = np.zeros((128, 4, 35), dWo.dtype)
    dwo_pad[:, :, 0:3] = dwo_t[:, :, 0:3]
    dwo_pad[:, :, 32:35] = dwo_t[:, :, 3:6]
    out["dwo"] = f(dwo_pad)
    db_all = np.stack([db0, dbh[0], dbh[1], dbh[2]], axis=0)  # [4, DH]
    out["dbias"] = f(db_all.reshape(4, 4, 128).transpose(2, 0, 1))  # [128,4,4]
    # rows 0:3 hold -bias (sigmoid via exp(-(x+b))), rows 32:35 hold +bias
    dbo_pad = np.zeros((35, 1), dbo.dtype)
    dbo_pad[0:3, 0] = -dbo[:3]
    dbo_pad[32:35, 0] = dbo[3:]
    out["dbo"] = f(dbo_pad)

    out["pw0i"] = f(pW0[:5])
    out["pw0z"] = f(pW0[5:])
    out["pwh"] = f(pWh[0].reshape(2, 128, HG).transpose(1, 0, 2))  # [128,2,HG]
    out["pwo"] = f(pWo.reshape(2, 128, 1).transpose(1, 0, 2))  # [128,2,1]
    pb_all = np.stack([pb0, pbh[0]], axis=0)  # [2, HG]
    out["pb"] = f(pb_all.reshape(2, 2, 128).transpose(2, 0, 1))  # [128,2,2]

    out["qw0i"] = f(qW0[:2])
    out["qw0z"] = f(qW0[2:])
    out["qwh"] = f(qWh[0].reshape(2, 128, HG).transpose(1, 0, 2))
    out["qwo"] = f(qWo.reshape(2, 128, 1).transpose(1, 0, 2))
    qb_all = np.stack([qb0, qbh[0]], axis=0)
    out["qb"] = f(qb_all.reshape(2, 2, 128).transpose(2, 0, 1))
    return {k2: np.asarray(v, np.float32) for k2, v in out.items()}


def kernel(**inputs):
    from concourse.bass_utils import run_bass_kernel_spmd
    import os

    inputs = {k2: np.asarray(v, np.float32) for k2, v in inputs.items()}
    if "nc" not in _CACHE:
        _CACHE["nc"] = _build()
    nc = _CACHE["nc"]

    shared = _prep_shared(inputs)
    x_c, y_c = inputs["x_c"], inputs["y_c"]
    x_t, y_t = inputs["x_t"], inputs["y_t"]
    x_pred, y_pred, eps = inputs["x_pred"], inputs["y_pred"], inputs["eps"]

    in_maps = []
    for b in range(B):
        m = dict(shared)
        memo_c = np.concatenate([x_c[b], y_c[b]], axis=-1).T  # [5, NC]
        memo_t = np.concatenate([x_t[b], y_t[b]], axis=-1).T  # [5, NT]
        m["enc_in"] = np.ascontiguousarray(
            np.stack([memo_c, memo_t], axis=1), np.float32
        )  # [5, 2, 512]
        m["xyT"] = np.ascontiguousarray(
            np.concatenate([x_pred[b], y_pred[b]], axis=-1).T, np.float32
        )  # [5, T]
        m["epsT"] = np.ascontiguousarray(eps[b].T, np.float32)  # [128, K]
        in_maps.append(m)

    trace = bool(int(os.environ.get("BASS_KERNEL_TRACE", "0")))
    if trace:
        try:
            import trnprof

            trnprof.install()
        except Exception:
            trace = False
    res = run_bass_kernel_spmd(nc, in_maps, core_ids=list(range(B)), trace=trace)
    _CACHE["exec_time_ns"] = res.exec_time_ns

    mu_c = np.empty((B, K, L), np.float32)
    lv_c = np.empty((B, K, L), np.float32)
    mu_t = np.empty((B, K, L), np.float32)
    lv_t = np.empty((B, K, L), np.float32)
    y_mean = np.empty((B, T, K, DY), np.float32)
    y_std = np.empty((B, T, K, DY), np.float32)
    alpha_post = np.empty((B, T, K), np.float32)
    alpha_prior = np.empty((B, T, K), np.float32)
    for b in range(B):
        r = res.results[b]
        eo = r["enc_out"]  # [128, 2, 2, K]
        mu_c[b] = eo[:, 0, 0, :].T
        lv_c[b] = eo[:, 0, 1, :].T
        mu_t[b] = eo[:, 1, 0, :].T
        lv_t[b] = eo[:, 1, 1, :].T
        do = r["dec_out"]  # [K, 6, T]
        y_mean[b] = do[:, 0:3, :].transpose(2, 0, 1)
        y_std[b] = do[:, 3:6, :].transpose(2, 0, 1)
        alpha_post[b] = r["alpha_post"]
        alpha_prior[b] = r["alpha_prior"]

    return (mu_c, lv_c, mu_t, lv_t, y_mean, y_std, alpha_post, alpha_prior)


# revision 35
# speedup vs baseline: 1.0128x; 1.0128x over previous
"""Trainium2 Bass kernel for the MoE Conditional Neural Process problem.

Contract: kernel(**inputs) takes the FULL unsharded inputs (as produced by
reference.setup_inputs()) and returns the FULL output tuple
(mu_c, lv_c, mu_t, lv_t, y_mean, y_std, alpha_post, alpha_prior).

Sharding: data-parallel over batch B=8 across the 8 NeuronCores (one batch
element per core). Weights are replicated. All distribution / gather is done
host-side in this file.

Device kernel design (per core, batch element b):
  - activations kept feature-major: [features on partitions, points on free]
  - all matmuls in float32r (tf32-class precision, full PE rate)
  - encoder: per expert k, 3-layer MLP on 512 points (x2 sets), mean-pool
    fused into the last relu via accum_out, then mu/lv heads (N=1 matmuls)
  - z = mu_t + eps * exp(0.5 lv_t) on-chip
  - decoder/gates: input concat(x, z_k) -> the z part is constant per k, so
    W0_z @ z_k is folded into a per-expert bias; the x projection is computed
    once (k-independent) and reused for all 8 experts
  - gate logits computed row-major ([128 rows, 1] matmuls with the hidden
    activations as the stationary operand), softmax over K on-chip
"""

import numpy as np

B, NC, NT, T = 8, 512, 512, 1024
DX, DY, L, K = 2, 3, 128, 8
H, NH = 512, 2
DH, NDH = 512, 3
HG, NG = 256, 1

_CACHE = {}


def _build():
    import concourse.mybir as mybir
    import concourse.tile as tile
    from concourse import bacc

    f32 = mybir.dt.float32
    f32r = mybir.dt.float32r
    AF = mybir.ActivationFunctionType
    ALU = mybir.AluOpType
    AX = mybir.AxisListType

    nc = bacc.Bacc(trn_type="TRN2", target_bir_lowering=False, debug=False)

    # ---------------- DRAM I/O ----------------
    d_enc_in = nc.dram_tensor("enc_in", [5, 2, 512], f32r, kind="ExternalInput")
    d_xyT = nc.dram_tensor("xyT", [5, T], f32r, kind="ExternalInput")
    d_epsT = nc.dram_tensor("epsT", [128, K], f32, kind="ExternalInput")

    d_ew0 = nc.dram_tensor("ew0", [K, 5, H], f32r, kind="ExternalInput")
    d_ewh = nc.dram_tensor("ewh", [K, 128, NH, 4, H], f32r, kind="ExternalInput")
    d_ewml = nc.dram_tensor("ewml", [K, 128, 4, 2 * L], f32r, kind="ExternalInput")
    d_ebias = nc.dram_tensor("ebias", [128, K, 3, 4], f32, kind="ExternalInput")
    d_emlb = nc.dram_tensor("emlb", [128, K, 2], f32, kind="ExternalInput")

    d_dw0x = nc.dram_tensor("dw0x", [2, DH], f32r, kind="ExternalInput")
    d_dw0z = nc.dram_tensor("dw0z", [128, DH], f32r, kind="ExternalInput")
    d_dwh = nc.dram_tensor("dwh", [128, NDH, 4, DH], f32r, kind="ExternalInput")
    d_dwo = nc.dram_tensor("dwo", [128, 4, 35], f32r, kind="ExternalInput")
    d_dbias = nc.dram_tensor("dbias", [128, 4, 4], f32, kind="ExternalInput")
    d_dbo = nc.dram_tensor("dbo", [35, 1], f32, kind="ExternalInput")

    d_pw0i = nc.dram_tensor("pw0i", [5, HG], f32r, kind="ExternalInput")
    d_pw0z = nc.dram_tensor("pw0z", [128, HG], f32r, kind="ExternalInput")
    d_pwh = nc.dram_tensor("pwh", [128, 2, HG], f32r, kind="ExternalInput")
    d_pwo = nc.dram_tensor("pwo", [128, 2, 1], f32r, kind="ExternalInput")
    d_pb = nc.dram_tensor("pb", [128, 2, 2], f32, kind="ExternalInput")

    d_qw0i = nc.dram_tensor("qw0i", [2, HG], f32r, kind="ExternalInput")
    d_qw0z = nc.dram_tensor("qw0z", [128, HG], f32r, kind="ExternalInput")
    d_qwh = nc.dram_tensor("qwh", [128, 2, HG], f32r, kind="ExternalInput")
    d_qwo = nc.dram_tensor("qwo", [128, 2, 1], f32r, kind="ExternalInput")
    d_qb = nc.dram_tensor("qb", [128, 2, 2], f32, kind="ExternalInput")

    d_lg = nc.dram_tensor("lg_scratch", [2, K, T], f32, kind="Internal")
    d_enc_out = nc.dram_tensor("enc_out", [128, 2, 2, K], f32, kind="ExternalOutput")
    d_dec_out = nc.dram_tensor("dec_out", [K, 6, T], f32, kind="ExternalOutput")
    d_ap_out = nc.dram_tensor("alpha_post", [T, K], f32, kind="ExternalOutput")
    d_aq_out = nc.dram_tensor("alpha_prior", [T, K], f32, kind="ExternalOutput")

    alt = [0]

    with tile.TileContext(nc) as tc:
        import contextlib

        with contextlib.ExitStack() as ctx:
            consts = ctx.enter_context(tc.tile_pool(name="consts", bufs=1))
            encw = ctx.enter_context(tc.tile_pool(name="encw", bufs=2))
            acts = ctx.enter_context(tc.tile_pool(name="acts", bufs=2))
            acts3 = ctx.enter_context(tc.tile_pool(name="acts3", bufs=2))
            # g0 lives across expert iterations: 4 in flight + 4 draining
            g0pool = ctx.enter_context(tc.tile_pool(name="g0pool", bufs=5))
            ghpool = ctx.enter_context(tc.tile_pool(name="ghpool", bufs=4))
            xproj = ctx.enter_context(tc.tile_pool(name="xproj", bufs=1))
            persist = ctx.enter_context(tc.tile_pool(name="persist", bufs=1))
            small = ctx.enter_context(tc.tile_pool(name="small", bufs=2))
            smx = ctx.enter_context(tc.tile_pool(name="smx", bufs=8))
            outs = ctx.enter_context(tc.tile_pool(name="outs", bufs=2))
            ps_h = ctx.enter_context(tc.tile_pool(name="ps_h", bufs=5, space="PSUM"))
            ps_s = ctx.enter_context(tc.tile_pool(name="ps_s", bufs=1, space="PSUM"))
            ps_l = ctx.enter_context(tc.tile_pool(name="ps_l", bufs=1, space="PSUM"))
            ps_o = ctx.enter_context(tc.tile_pool(name="ps_o", bufs=1, space="PSUM"))

            zcol = None

            def relu_store(out, in_, bias, accum_out=None, sbuf_src=False):
                """relu(in_ + bias) -> out, alternating ACT / DVE.

                NB: tensor_scalar with scalar1=AP and scalar2=immediate
                silently drops op1 on this toolchain; scalar2 must be an AP.
                """
                alt[0] ^= 1
                if alt[0] or accum_out is not None:
                    # DVE tensor_scalar with accum_out corrupts both outputs
                    # on this toolchain -- keep accumulating relus on ACT
                    nc.scalar.activation(
                        out=out, in_=in_, func=AF.Relu, bias=bias, accum_out=accum_out
                    )
                else:
                    nc.vector.tensor_scalar(
                        out, in_, bias, zcol[:, 0:1], ALU.add, ALU.max
                    )

            def mm32(ps, lhsT, rhs, start, stop):
                # N=1 matmuls: fp32r rejects free-size-1 moving operands in
                # codegen; run them as plain fp32 (cost is negligible at N=1)
                nc.tensor.matmul(
                    ps, lhsT.bitcast(f32), rhs.bitcast(f32), start=start, stop=stop
                )

            def load_enc(k):
                t0 = encw.tile([5, H], f32r, tag="ew0")
                nc.sync.dma_start(t0[:], d_ew0[k])
                t1 = encw.tile([128, NH, 4, H], f32r, tag="ewh")
                nc.sync.dma_start(t1[:], d_ewh[k])
                t2 = encw.tile([128, 4, 2 * L], f32r, tag="ewml")
                nc.sync.dma_start(t2[:], d_ewml[k])
                return t0, t1, t2

            # ---------------- constant loads ----------------
            # order matters: the Sync DMA queue drains in order, so small
            # tensors needed by the first compute (xproj, encoder L0) go
            # first, then the k=0 encoder weights (2.4 MB), then the rest
            zcol = consts.tile([128, 1], f32)
            nc.vector.memset(zcol[:], 0.0)
            xyT = consts.tile([5, T], f32r)
            nc.sync.dma_start(xyT[:], d_xyT[:])
            xT = xyT[0:2, :]
            dw0x = consts.tile([2, DH], f32r)
            nc.sync.dma_start(dw0x[:], d_dw0x[:])
            pw0i = consts.tile([5, HG], f32r)
            nc.sync.dma_start(pw0i[:], d_pw0i[:])
            qw0i = consts.tile([2, HG], f32r)
            nc.sync.dma_start(qw0i[:], d_qw0i[:])
            enc_in = consts.tile([5, 2, 512], f32r)
            nc.sync.dma_start(enc_in[:], d_enc_in[:])
            ebias = consts.tile([128, K, 3, 4], f32)
            nc.sync.dma_start(ebias[:], d_ebias[:])

            pending_encw = load_enc(0)

            epsT = consts.tile([128, K], f32)
            nc.sync.dma_start(epsT[:], d_epsT[:])
            emlb = consts.tile([128, K, 2], f32)
            nc.sync.dma_start(emlb[:], d_emlb[:])
            dbias = consts.tile([128, 4, 4], f32)
            nc.sync.dma_start(dbias[:], d_dbias[:])
            dbo = consts.tile([35, 1], f32)
            nc.sync.dma_start(dbo[:], d_dbo[:])
            pb = consts.tile([128, 2, 2], f32)
            nc.sync.dma_start(pb[:], d_pb[:])
            qb = consts.tile([128, 2, 2], f32)
            nc.sync.dma_start(qb[:], d_qb[:])

            # persistent state
            zT = persist.tile([128, K], f32r)
            enc_sb = persist.tile([128, 2, 2, K], f32)
            asb_p = persist.tile([128, 8, K], f32)
            asb_q = persist.tile([128, 8, K], f32)

            # ---------------- x projections (k-independent) ----------------
            xpd = xproj.tile([128, 4, T], f32)
            xpp = xproj.tile([128, 2, T], f32)
            xpq = xproj.tile([128, 2, T], f32)
            for wsb, xin, n_o, xp in (
                (dw0x, xT, 4, xpd),
                (pw0i, xyT, 2, xpp),
                (qw0i, xT, 2, xpq),
            ):
                for c in range(2):
                    for o in range(n_o):
                        ps = ps_h.tile([128, 512], f32, tag="ph")
                        nc.tensor.matmul(
                            ps[:],
                            wsb[:, o * 128 : (o + 1) * 128],
                            xin[:, c * 512 : (c + 1) * 512],
                            start=True,
                            stop=True,
                        )
                        nc.vector.tensor_copy(xp[:, o, c * 512 : (c + 1) * 512], ps[:])

            # ---------------- encoders ----------------
            for k in range(K):
                ew0_k, ewh_k, ewml_k = pending_encw
                if k + 1 < K:
                    pending_encw = load_enc(k + 1)
                rr_all = small.tile([128, 4, 2], f32r, tag="rr")

                for s in range(2):
                    h = acts.tile([128, 4, 512], f32r, tag="ench")
                    for o in range(4):
                        ps = ps_h.tile([128, 512], f32, tag="ph")
                        nc.tensor.matmul(
                            ps[:],
                            ew0_k[:, o * 128 : (o + 1) * 128],
                            enc_in[:, s, :],
                            start=True,
                            stop=True,
                        )
                        relu_store(h[:, o, :], ps[:], ebias[:, k, 0, o : o + 1])
                    rsum = small.tile([128, 4], f32, tag="rsum")
                    for l in (1, 2):
                        hn = acts.tile([128, 4, 512], f32r, tag="ench")
                        for o in range(4):
                            ps = ps_h.tile([128, 512], f32, tag="ph")
                            for i in range(4):
                                nc.tensor.matmul(
                                    ps[:],
                                    ewh_k[:, l - 1, i, o * 128 : (o + 1) * 128],
                                    h[:, i, :],
                                    start=(i == 0),
                                    stop=(i == 3),
                                )
                            relu_store(
                                hn[:, o, :],
                                ps[:],
                                ebias[:, k, l, o : o + 1],
                                accum_out=(rsum[:, o : o + 1] if l == 2 else None),
                            )
                        h = hn
                    nc.vector.tensor_copy(rr_all[:, :, s], rsum[:])
                # mu/lv heads for both sets at once (N=2); the 1/512 mean
                # factor is folded into the bias-add below
                ps_mu = ps_s.tile([128, 2], f32, tag="pss")
                for i in range(4):
                    nc.tensor.matmul(
                        ps_mu[:], ewml_k[:, i, 0:L], rr_all[:, i, :],
                        start=(i == 0), stop=(i == 3),
                    )
                ps_lv = ps_s.tile([128, 2], f32, tag="pss")
                for i in range(4):
                    nc.tensor.matmul(
                        ps_lv[:], ewml_k[:, i, L : 2 * L], rr_all[:, i, :],
                        start=(i == 0), stop=(i == 3),
                    )
                for s in range(2):
                    nc.vector.scalar_tensor_tensor(
                        out=enc_sb[:, s, 0, k : k + 1],
                        in0=ps_mu[:, s : s + 1],
                        scalar=1.0 / 512.0,
                        in1=emlb[:, k, 0:1],
                        op0=ALU.mult,
                        op1=ALU.add,
                    )
                    nc.vector.scalar_tensor_tensor(
                        out=enc_sb[:, s, 1, k : k + 1],
                        in0=ps_lv[:, s : s + 1],
                        scalar=1.0 / 512.0,
                        in1=emlb[:, k, 1:2],
                        op0=ALU.mult,
                        op1=ALU.add,
                    )
                ze = small.tile([128, 1], f32, tag="ze")
                nc.scalar.activation(
                    out=ze[:],
                    in_=enc_sb[:, 1, 1, k : k + 1],
                    func=AF.Exp,
                    scale=0.5,
                )
                zm = small.tile([128, 1], f32, tag="zm")
                nc.vector.tensor_mul(zm[:], ze[:], epsT[:, k : k + 1])
                nc.vector.tensor_add(
                    zT[:, k : k + 1], zm[:], enc_sb[:, 1, 0, k : k + 1]
                )
            nc.sync.dma_start(d_enc_out[:], enc_sb[:])

            # phase-D weights: loaded after the encoder weights so the
            # startup DMA queue serves the encoder first (these are not
            # needed until the encoder phase is done)
            dw0z = consts.tile([128, DH], f32r)
            nc.sync.dma_start(dw0z[:], d_dw0z[:])
            dwh = consts.tile([128, NDH, 4, DH], f32r)
            nc.sync.dma_start(dwh[:], d_dwh[:])
            dwo = consts.tile([128, 4, 35], f32r)
            nc.sync.dma_start(dwo[:], d_dwo[:])
            pw0z = consts.tile([128, HG], f32r)
            nc.sync.dma_start(pw0z[:], d_pw0z[:])
            pwh = consts.tile([128, 2, HG], f32r)
            nc.sync.dma_start(pwh[:], d_pwh[:])
            pwo = consts.tile([128, 2, 1], f32r)
            nc.sync.dma_start(pwo[:], d_pwo[:])
            qw0z = consts.tile([128, HG], f32r)
            nc.sync.dma_start(qw0z[:], d_qw0z[:])
            qwh = consts.tile([128, 2, HG], f32r)
            nc.sync.dma_start(qwh[:], d_qwh[:])
            qwo = consts.tile([128, 2, 1], f32r)
            nc.sync.dma_start(qwo[:], d_qwo[:])
            # ---------------- z-dependent biases, all experts at once ----
            # zball slots: 0..3 decoder o-tiles, 4..5 post gate, 6..7 prior
            zball = persist.tile([128, 8, K], f32)
            zb_specs = (
                [(dw0z, o, dbias[:, 0, o : o + 1], o) for o in range(4)]
                + [(pw0z, o, pb[:, 0, o : o + 1], 4 + o) for o in range(2)]
                + [(qw0z, o, qb[:, 0, o : o + 1], 6 + o) for o in range(2)]
            )
            for zw, o, bias_ap, slot in zb_specs:
                ps = ps_s.tile([128, K], f32, tag="pss")
                nc.tensor.matmul(
                    ps[:], zw[:, o * 128 : (o + 1) * 128], zT[:], start=True, stop=True
                )
                nc.vector.tensor_scalar(
                    zball[:, slot, :], ps[:], bias_ap, zcol[:, 0:1], ALU.add, ALU.add
                )

            # ---------------- per-expert gates + decoder ----------------
            def emit_g0(k):
                # gate L0 relus for expert k; emitted one expert ahead so the
                # ACT/DVE engines produce them while the PE runs the previous
                # expert's decoder (kills the PE stall at expert boundaries)
                res = []
                for xp, zslice in (
                    (xpp, zball[:, 4:6, k : k + 1]),
                    (xpq, zball[:, 6:8, k : k + 1]),
                ):
                    per_c = []
                    for c in range(2):
                        g0 = g0pool.tile([128, 2, 512], f32r, tag="g0")
                        for o in range(2):
                            relu_store(
                                g0[:, o, :],
                                xp[:, o, c * 512 : (c + 1) * 512],
                                zslice[:, o, :],
                                sbuf_src=True,
                            )
                        per_c.append(g0)
                    res.append(per_c)
                return res

            def gate_stage1(k, gi):
                # gate hidden layer (PE + relu); g0 was produced during the
                # previous expert's decoder
                wh, gb = ((pwh, pb), (qwh, qb))[gi]
                per_c = []
                for c in range(2):
                    g0 = g0_cur[gi][c]
                    g1 = ghpool.tile([128, 2, 512], f32r, tag="gh")
                    for o in range(2):
                        ps = ps_h.tile([128, 512], f32, tag="ph")
                        for i in range(2):
                            nc.tensor.matmul(
                                ps[:],
                                wh[:, i, o * 128 : (o + 1) * 128],
                                g0[:, i, :],
                                start=(i == 0),
                                stop=(i == 1),
                            )
                        relu_store(g1[:, o, :], ps[:], gb[:, 1, o : o + 1])
                    per_c.append(g1)
                return per_c

            def gate_stage2(k, gi, g1s):
                # logits (weight-stationary, M=1) + softmax-layout scatter
                wo = (pwo, qwo)[gi]
                asb = (asb_p, asb_q)[gi]
                for c in range(2):
                    psl = ps_l.tile([1, 512], f32, tag="psl")
                    for i in range(2):
                        nc.tensor.matmul(
                            psl[:],
                            wo[:, i, 0:1],
                            g1s[c][:, i, :],
                            start=(i == 0),
                            stop=(i == 1),
                        )
                    lgc = outs.tile([1, 512], f32, tag="lg")
                    nc.vector.tensor_copy(lgc[:], psl[:])
                    nc.sync.dma_start(d_lg[gi, k, c * 512 : (c + 1) * 512], lgc[:])
                nc.sync.dma_start(
                    asb[:, :, k],
                    d_lg[gi, k].rearrange("(c p) -> p c", p=128),
                )

            def emit_softmax():
                for asb, dout in ((asb_p, d_ap_out), (asb_q, d_aq_out)):
                    for r in range(8):
                        nm = smx.tile([128, 1], f32, tag="sm")
                        nc.vector.tensor_reduce(
                            out=nm[:], in_=asb[:, r, :], axis=AX.X, op=ALU.max
                        )
                        nc.scalar.mul(nm[:], nm[:], -1.0)
                        e = smx.tile([128, K], f32, tag="se")
                        nc.scalar.activation(
                            out=e[:], in_=asb[:, r, :], func=AF.Exp, bias=nm[:, 0:1]
                        )
                        ssum = smx.tile([128, 1], f32, tag="ss")
                        nc.vector.tensor_reduce(
                            out=ssum[:], in_=e[:], axis=AX.X, op=ALU.add
                        )
                        rec = smx.tile([128, 1], f32, tag="sr")
                        nc.vector.reciprocal_approx_fast(out=rec[:], in_=ssum[:])
                        a = smx.tile([128, K], f32, tag="sa")
                        nc.vector.tensor_scalar_mul(a[:], e[:], rec[:, 0:1])
                        nc.sync.dma_start(dout[r * 128 : (r + 1) * 128, :], a[:])

            # the gate pipeline for expert k is spread through the decoder of
            # expert k, stage by stage, so the PE stream never drains on the
            # gate latency chain (matmul -> relu -> matmul -> copy)
            g0_cur = emit_g0(0)
            g1_cur = [gate_stage1(0, 0), gate_stage1(0, 1)]
            for k in range(K):
                zb_d = zball[:, 0:4, k : k + 1]
                ymo = outs.tile([35, T], f32, tag="ymo")
                for c in range(2):
                    h = acts3.tile([128, 4, 512], f32r, tag="dech")
                    for o in range(4):
                        relu_store(
                            h[:, o, :],
                            xpd[:, o, c * 512 : (c + 1) * 512],
                            zb_d[:, o, :],
                            sbuf_src=True,
                        )
                    for l in range(NDH):
                        hn = acts3.tile([128, 4, 512], f32r, tag="dech")
                        for o in range(4):
                            ps = ps_h.tile([128, 512], f32, tag="ph")
                            for i in range(4):
                                nc.tensor.matmul(
                                    ps[:],
                                    dwh[:, l, i, o * 128 : (o + 1) * 128],
                                    h[:, i, :],
                                    start=(i == 0),
                                    stop=(i == 3),
                                )
                            relu_store(hn[:, o, :], ps[:], dbias[:, l + 1, o : o + 1])
                        h = hn
                        if c == 0 and l == 0:
                            gate_stage2(k, 0, g1_cur[0])
                        elif c == 0 and l == 1:
                            gate_stage2(k, 1, g1_cur[1])
                            if k == K - 1:
                                emit_softmax()
                        elif c == 0 and l == 2 and k + 1 < K:
                            g0_cur = emit_g0(k + 1)
                        elif c == 1 and l == 0 and k + 1 < K:
                            g1_cur[0] = gate_stage1(k + 1, 0)
                        elif c == 1 and l == 1 and k + 1 < K:
                            g1_cur[1] = gate_stage1(k + 1, 1)
                    pso = ps_o.tile([35, 512], f32, tag="po")
                    for i in range(4):
                        nc.tensor.matmul(
                            pso[:],
                            dwo[:, i, :],
                            h[:, i, :],
                            start=(i == 0),
                            stop=(i == 3),
                        )
                    # sigmoid(x+b) = 1/(1+exp(-(x+b)));  dbo holds -b in rows
                    # 0:3 and +b in rows 3:6 (prepared host-side)
                    et = small.tile([35, 512], f32, tag="eo")
                    nc.scalar.activation(
                        out=et[0:3, :],
                        in_=pso[0:3, :],
                        func=AF.Exp,
                        bias=dbo[0:3, 0:1],
                        scale=-1.0,
                    )
                    nc.scalar.activation(
                        out=et[32:35, :],
                        in_=pso[32:35, :],
                        func=AF.Exp,
                        bias=dbo[32:35, 0:1],
                    )
                    nc.vector.tensor_scalar_add(et[:], et[:], 1.0)
                    nc.vector.reciprocal_approx_fast(
                        out=ymo[0:3, c * 512 : (c + 1) * 512], in_=et[0:3, :]
                    )
                    # softplus(x+b) = ln(1 + exp(x+b))
                    nc.scalar.activation(
                        out=ymo[32:35, c * 512 : (c + 1) * 512],
                        in_=et[32:35, :],
                        func=AF.Ln,
                    )
                nc.sync.dma_start(d_dec_out[k, 0:3], ymo[0:3, :])
                nc.sync.dma_start(d_dec_out[k, 3:6], ymo[32:35, :])


    nc.compile()
    return nc


def _prep_shared(inp):
    """Host-side weight layout transforms (same for all cores)."""
    f = np.ascontiguousarray
    eW0, eb0, eWh, ebh = inp["eW0"], inp["eb0"], inp["eWh"], inp["ebh"]
    eWmu, ebmu, eWlv, eblv = inp["eWmu"], inp["ebmu"], inp["eWlv"], inp["eblv"]
    pW0, pb0, pWh, pbh, pWo = inp["pW0"], inp["pb0"], inp["pWh"], inp["pbh"], inp["pWo"]
    qW0, qb0, qWh, qbh, qWo = inp["qW0"], inp["qb0"], inp["qWh"], inp["qbh"], inp["qWo"]
    dW0, db0, dWh, dbh, dWo, dbo = (
        inp["dW0"], inp["db0"], inp["dWh"], inp["dbh"], inp["dWo"], inp["dbo"],
    )

    out = {}
    out["ew0"] = f(eW0)  # [K, 5, H]
    # ewh[k, p, l, i, o] = eWh[l, k, i*128+p, o]
    ewh = eWh.reshape(NH, K, 4, 128, H).transpose(1, 3, 0, 2, 4)
    out["ewh"] = f(ewh)
    # ewml[k, p, i, :128]=eWmu[k, i*128+p, :], [128:]=eWlv
    wmu = eWmu.reshape(K, 4, 128, L).transpose(0, 2, 1, 3)
    wlv = eWlv.reshape(K, 4, 128, L).transpose(0, 2, 1, 3)
    out["ewml"] = f(np.concatenate([wmu, wlv], axis=-1))  # [K,128,4,256]
    # ebias[p, k, l, o]: l=0 -> eb0[k, o*128+p]; l=1,2 -> ebh[l-1, k, o*128+p]
    eb_all = np.stack([eb0, ebh[0], ebh[1]], axis=1)  # [K, 3, H]
    out["ebias"] = f(eb_all.reshape(K, 3, 4, 128).transpose(3, 0, 1, 2))
    out["emlb"] = f(np.stack([ebmu, eblv], axis=-1).transpose(1, 0, 2))  # [128,K,2]

    out["dw0x"] = f(dW0[:2])  # [2, DH]
    out["dw0z"] = f(dW0[2:])  # [128, DH]
    out["dwh"] = f(dWh.reshape(NDH, 4, 128, DH).transpose(2, 0, 1, 3))  # [128,3,4,DH]
    dwo_t = dWo.reshape(4, 128, 6).transpose(1, 0, 2)  # [128,4,6]
    dwo_pad = np.zeros((128, 4, 35), dWo.dtype)
    dwo_pad[:, :, 0:3] = dwo_t[:, :, 0:3]
    dwo_pad[:, :, 32:35] = dwo_t[:, :, 3:6]
    out["dwo"] = f(dwo_pad)
    db_all = np.stack([db0, dbh[0], dbh[1], dbh[2]], axis=0)  # [4, DH]
    out["dbias"] = f(db_all.reshape(4, 4, 128).transpose(2, 0, 1))  # [128,4,4]
    # rows 0:3 hold -bias (sigmoid via exp(-(x+b))), rows 32:35 hold +bias
    dbo_pad = np.zeros((35, 1), dbo.dtype)
    dbo_pad[0:3, 0] = -dbo[:3]
    dbo_pad[32:35, 0] = dbo[3:]
    out["dbo"] = f(dbo_pad)

    out["pw0i"] = f(pW0[:5])
    out["pw0z"] = f(pW0[5:])
    out["pwh"] = f(pWh[0].reshape(2, 128, HG).transpose(1, 0, 2))  # [128,2,HG]
    out["pwo"] = f(pWo.reshape(2, 128, 1).transpose(1, 0, 2))  # [128,2,1]
    pb_all = np.stack([pb0, pbh[0]], axis=0)  # [2, HG]
    out["pb"] = f(pb_all.reshape(2, 2, 128).transpose(2, 0, 1))  # [128,2,2]

    out["qw0i"] = f(qW0[:2])
    out["qw0z"] = f(qW0[2:])
    out["qwh"] = f(qWh[0].reshape(2, 128, HG).transpose(1, 0, 2))
    out["qwo"] = f(qWo.reshape(2, 128, 1).transpose(1, 0, 2))
    qb_all = np.stack([qb0, qbh[0]], axis=0)
    out["qb"] = f(qb_all.reshape(2, 2, 128).transpose(2, 0, 1))
    return {k2: np.asarray(v, np.float32) for k2, v in out.items()}


def kernel(**inputs):
    from concourse.bass_utils import run_bass_kernel_spmd
    import os

    inputs = {k2: np.asarray(v, np.float32) for k2, v in inputs.items()}
    if "nc" not in _CACHE:
        _CACHE["nc"] = _build()
    nc = _CACHE["nc"]

    shared = _prep_shared(inputs)
    x_c, y_c = inputs["x_c"], inputs["y_c"]
    x_t, y_t = inputs["x_t"], inputs["y_t"]
    x_pred, y_pred, eps = inputs["x_pred"], inputs["y_pred"], inputs["eps"]

    in_maps = []
    for b in range(B):
        m = dict(shared)
        memo_c = np.concatenate([x_c[b], y_c[b]], axis=-1).T  # [5, NC]
        memo_t = np.concatenate([x_t[b], y_t[b]], axis=-1).T  # [5, NT]
        m["enc_in"] = np.ascontiguousarray(
            np.stack([memo_c, memo_t], axis=1), np.float32
        )  # [5, 2, 512]
        m["xyT"] = np.ascontiguousarray(
            np.concatenate([x_pred[b], y_pred[b]], axis=-1).T, np.float32
        )  # [5, T]
        m["epsT"] = np.ascontiguousarray(eps[b].T, np.float32)  # [128, K]
        in_maps.append(m)

    trace = bool(int(os.environ.get("BASS_KERNEL_TRACE", "0")))
    if trace:
        try:
            import trnprof

            trnprof.install()
        except Exception:
            trace = False
    res = run_bass_kernel_spmd(nc, in_maps, core_ids=list(range(B)), trace=trace)
    _CACHE["exec_time_ns"] = res.exec_time_ns

    mu_c = np.empty((B, K, L), np.float32)
    lv_c = np.empty((B, K, L), np.float32)
    mu_t = np.empty((B, K, L), np.float32)
    lv_t = np.empty((B, K, L), np.float32)
    y_mean = np.empty((B, T, K, DY), np.float32)
    y_std = np.empty((B, T, K, DY), np.float32)
    alpha_post = np.empty((B, T, K), np.float32)
    alpha_prior = np.empty((B, T, K), np.float32)
    for b in range(B):
        r = res.results[b]
        eo = r["enc_out"]  # [128, 2, 2, K]
        mu_c[b] = eo[:, 0, 0, :].T
        lv_c[b] = eo[:, 0, 1, :].T
        mu_t[b] = eo[:, 1, 0, :].T
        lv_t[b] = eo[:, 1, 1, :].T
        do = r["dec_out"]  # [K, 6, T]
        y_mean[b] = do[:, 0:3, :].transpose(2, 0, 1)
        y_std[b] = do[:, 3:6, :].transpose(2, 0, 1)
        alpha_post[b] = r["alpha_post"]
        alpha_prior[b] = r["alpha_prior"]

    return (mu_c, lv_c, mu_t, lv_t, y_mean, y_std, alpha_post, alpha_prior)


# revision 36
# speedup vs baseline: 1.0278x; 1.0149x over previous
"""Trainium2 Bass kernel for the MoE Conditional Neural Process problem.

Contract: kernel(**inputs) takes the FULL unsharded inputs (as produced by
reference.setup_inputs()) and returns the FULL output tuple
(mu_c, lv_c, mu_t, lv_t, y_mean, y_std, alpha_post, alpha_prior).

Sharding: data-parallel over batch B=8 across the 8 NeuronCores (one batch
element per core). Weights are replicated. All distribution / gather is done
host-side in this file.

Device kernel design (per core, batch element b):
  - activations kept feature-major: [features on partitions, points on free]
  - all matmuls in float32r (tf32-class precision, full PE rate)
  - encoder: per expert k, 3-layer MLP on 512 points (x2 sets), mean-pool
    fused into the last relu via accum_out, then mu/lv heads (N=1 matmuls)
  - z = mu_t + eps * exp(0.5 lv_t) on-chip
  - decoder/gates: input concat(x, z_k) -> the z part is constant per k, so
    W0_z @ z_k is folded into a per-expert bias; the x projection is computed
    once (k-independent) and reused for all 8 experts
  - gate logits computed row-major ([128 rows, 1] matmuls with the hidden
    activations as the stationary operand), softmax over K on-chip
"""

import numpy as np

B, NC, NT, T = 8, 512, 512, 1024
DX, DY, L, K = 2, 3, 128, 8
H, NH = 512, 2
DH, NDH = 512, 3
HG, NG = 256, 1

_CACHE = {}


def _build():
    import concourse.mybir as mybir
    import concourse.tile as tile
    from concourse import bacc

    f32 = mybir.dt.float32
    f32r = mybir.dt.float32r
    AF = mybir.ActivationFunctionType
    ALU = mybir.AluOpType
    AX = mybir.AxisListType

    nc = bacc.Bacc(trn_type="TRN2", target_bir_lowering=False, debug=False)

    # ---------------- DRAM I/O ----------------
    d_enc_in = nc.dram_tensor("enc_in", [5, 2, 512], f32r, kind="ExternalInput")
    d_xyT = nc.dram_tensor("xyT", [5, T], f32r, kind="ExternalInput")
    d_epsT = nc.dram_tensor("epsT", [128, K], f32, kind="ExternalInput")

    d_ew0 = nc.dram_tensor("ew0", [K, 5, H], f32r, kind="ExternalInput")
    d_ewh = nc.dram_tensor("ewh", [K, 128, NH, 4, H], f32r, kind="ExternalInput")
    d_ewml = nc.dram_tensor("ewml", [K, 128, 4, 2 * L], f32r, kind="ExternalInput")
    d_ebias = nc.dram_tensor("ebias", [128, K, 3, 4], f32, kind="ExternalInput")
    d_emlb = nc.dram_tensor("emlb", [128, K, 2], f32, kind="ExternalInput")

    d_dw0x = nc.dram_tensor("dw0x", [2, DH], f32r, kind="ExternalInput")
    d_dw0z = nc.dram_tensor("dw0z", [128, DH], f32r, kind="ExternalInput")
    d_dwh = nc.dram_tensor("dwh", [128, NDH, 4, DH], f32r, kind="ExternalInput")
    d_dwo = nc.dram_tensor("dwo", [128, 4, 35], f32r, kind="ExternalInput")
    d_dbias = nc.dram_tensor("dbias", [128, 4, 4], f32, kind="ExternalInput")
    d_dbo = nc.dram_tensor("dbo", [35, 1], f32, kind="ExternalInput")

    d_pw0i = nc.dram_tensor("pw0i", [5, HG], f32r, kind="ExternalInput")
    d_pw0z = nc.dram_tensor("pw0z", [128, HG], f32r, kind="ExternalInput")
    d_pwh = nc.dram_tensor("pwh", [128, 2, HG], f32r, kind="ExternalInput")
    d_pwo = nc.dram_tensor("pwo", [128, 2, 1], f32r, kind="ExternalInput")
    d_pb = nc.dram_tensor("pb", [128, 2, 2], f32, kind="ExternalInput")

    d_qw0i = nc.dram_tensor("qw0i", [2, HG], f32r, kind="ExternalInput")
    d_qw0z = nc.dram_tensor("qw0z", [128, HG], f32r, kind="ExternalInput")
    d_qwh = nc.dram_tensor("qwh", [128, 2, HG], f32r, kind="ExternalInput")
    d_qwo = nc.dram_tensor("qwo", [128, 2, 1], f32r, kind="ExternalInput")
    d_qb = nc.dram_tensor("qb", [128, 2, 2], f32, kind="ExternalInput")

    d_lg = nc.dram_tensor("lg_scratch", [2, K, T], f32, kind="Internal")
    d_enc_out = nc.dram_tensor("enc_out", [128, 2, 2, K], f32, kind="ExternalOutput")
    d_dec_out = nc.dram_tensor("dec_out", [K, 6, T], f32, kind="ExternalOutput")
    d_ap_out = nc.dram_tensor("alpha_post", [T, K], f32, kind="ExternalOutput")
    d_aq_out = nc.dram_tensor("alpha_prior", [T, K], f32, kind="ExternalOutput")

    alt = [0]

    with tile.TileContext(nc) as tc:
        import contextlib

        with contextlib.ExitStack() as ctx:
            consts = ctx.enter_context(tc.tile_pool(name="consts", bufs=1))
            encw = ctx.enter_context(tc.tile_pool(name="encw", bufs=2))
            acts = ctx.enter_context(tc.tile_pool(name="acts", bufs=2))
            acts3 = ctx.enter_context(tc.tile_pool(name="acts3", bufs=2))
            # g0 lives across expert iterations: 4 in flight + 4 draining
            g0pool = ctx.enter_context(tc.tile_pool(name="g0pool", bufs=5))
            ghpool = ctx.enter_context(tc.tile_pool(name="ghpool", bufs=4))
            xproj = ctx.enter_context(tc.tile_pool(name="xproj", bufs=1))
            persist = ctx.enter_context(tc.tile_pool(name="persist", bufs=1))
            small = ctx.enter_context(tc.tile_pool(name="small", bufs=2))
            smx = ctx.enter_context(tc.tile_pool(name="smx", bufs=8))
            outs = ctx.enter_context(tc.tile_pool(name="outs", bufs=2))
            ps_h = ctx.enter_context(tc.tile_pool(name="ps_h", bufs=4, space="PSUM"))
            ps_s = ctx.enter_context(tc.tile_pool(name="ps_s", bufs=1, space="PSUM"))
            ps_l = ctx.enter_context(tc.tile_pool(name="ps_l", bufs=1, space="PSUM"))
            ps_o = ctx.enter_context(tc.tile_pool(name="ps_o", bufs=2, space="PSUM"))

            zcol = None

            def relu_store(out, in_, bias, accum_out=None, sbuf_src=False):
                """relu(in_ + bias) -> out, alternating ACT / DVE.

                NB: tensor_scalar with scalar1=AP and scalar2=immediate
                silently drops op1 on this toolchain; scalar2 must be an AP.
                """
                alt[0] ^= 1
                if alt[0] or accum_out is not None:
                    # DVE tensor_scalar with accum_out corrupts both outputs
                    # on this toolchain -- keep accumulating relus on ACT
                    nc.scalar.activation(
                        out=out, in_=in_, func=AF.Relu, bias=bias, accum_out=accum_out
                    )
                else:
                    nc.vector.tensor_scalar(
                        out, in_, bias, zcol[:, 0:1], ALU.add, ALU.max
                    )

            def mm32(ps, lhsT, rhs, start, stop):
                # N=1 matmuls: fp32r rejects free-size-1 moving operands in
                # codegen; run them as plain fp32 (cost is negligible at N=1)
                nc.tensor.matmul(
                    ps, lhsT.bitcast(f32), rhs.bitcast(f32), start=start, stop=stop
                )

            def load_enc(k):
                t0 = encw.tile([5, H], f32r, tag="ew0")
                nc.sync.dma_start(t0[:], d_ew0[k])
                t1 = encw.tile([128, NH, 4, H], f32r, tag="ewh")
                nc.sync.dma_start(t1[:], d_ewh[k])
                t2 = encw.tile([128, 4, 2 * L], f32r, tag="ewml")
                nc.sync.dma_start(t2[:], d_ewml[k])
                return t0, t1, t2

            # ---------------- constant loads ----------------
            # order matters: the Sync DMA queue drains in order, so small
            # tensors needed by the first compute (xproj, encoder L0) go
            # first, then the k=0 encoder weights (2.4 MB), then the rest
            zcol = consts.tile([128, 1], f32)
            nc.vector.memset(zcol[:], 0.0)
            xyT = consts.tile([5, T], f32r)
            nc.sync.dma_start(xyT[:], d_xyT[:])
            xT = xyT[0:2, :]
            dw0x = consts.tile([2, DH], f32r)
            nc.sync.dma_start(dw0x[:], d_dw0x[:])
            pw0i = consts.tile([5, HG], f32r)
            nc.sync.dma_start(pw0i[:], d_pw0i[:])
            qw0i = consts.tile([2, HG], f32r)
            nc.sync.dma_start(qw0i[:], d_qw0i[:])
            enc_in = consts.tile([5, 2, 512], f32r)
            nc.sync.dma_start(enc_in[:], d_enc_in[:])
            ebias = consts.tile([128, K, 3, 4], f32)
            nc.sync.dma_start(ebias[:], d_ebias[:])

            pending_encw = load_enc(0)

            epsT = consts.tile([128, K], f32)
            nc.sync.dma_start(epsT[:], d_epsT[:])
            emlb = consts.tile([128, K, 2], f32)
            nc.sync.dma_start(emlb[:], d_emlb[:])
            dbias = consts.tile([128, 4, 4], f32)
            nc.sync.dma_start(dbias[:], d_dbias[:])
            dbo = consts.tile([35, 1], f32)
            nc.sync.dma_start(dbo[:], d_dbo[:])
            pb = consts.tile([128, 2, 2], f32)
            nc.sync.dma_start(pb[:], d_pb[:])
            qb = consts.tile([128, 2, 2], f32)
            nc.sync.dma_start(qb[:], d_qb[:])

            # persistent state
            zT = persist.tile([128, K], f32r)
            enc_sb = persist.tile([128, 2, 2, K], f32)
            asb_p = persist.tile([128, 8, K], f32)
            asb_q = persist.tile([128, 8, K], f32)

            # ---------------- x projections (k-independent) ----------------
            xpd = xproj.tile([128, 4, T], f32)
            xpp = xproj.tile([128, 2, T], f32)
            xpq = xproj.tile([128, 2, T], f32)
            for wsb, xin, n_o, xp in (
                (dw0x, xT, 4, xpd),
                (pw0i, xyT, 2, xpp),
                (qw0i, xT, 2, xpq),
            ):
                for c in range(2):
                    for o in range(n_o):
                        ps = ps_h.tile([128, 512], f32, tag="ph")
                        nc.tensor.matmul(
                            ps[:],
                            wsb[:, o * 128 : (o + 1) * 128],
                            xin[:, c * 512 : (c + 1) * 512],
                            start=True,
                            stop=True,
                        )
                        nc.vector.tensor_copy(xp[:, o, c * 512 : (c + 1) * 512], ps[:])

            # ---------------- encoders ----------------
            for k in range(K):
                ew0_k, ewh_k, ewml_k = pending_encw
                if k + 1 < K:
                    pending_encw = load_enc(k + 1)
                rr_all = small.tile([128, 4, 2], f32r, tag="rr")

                for s in range(2):
                    h = acts.tile([128, 4, 512], f32r, tag="ench")
                    for o in range(4):
                        ps = ps_h.tile([128, 512], f32, tag="ph")
                        nc.tensor.matmul(
                            ps[:],
                            ew0_k[:, o * 128 : (o + 1) * 128],
                            enc_in[:, s, :],
                            start=True,
                            stop=True,
                        )
                        relu_store(h[:, o, :], ps[:], ebias[:, k, 0, o : o + 1])
                    rsum = small.tile([128, 4], f32, tag="rsum")
                    for l in (1, 2):
                        hn = acts.tile([128, 4, 512], f32r, tag="ench")
                        for o in range(4):
                            ps = ps_h.tile([128, 512], f32, tag="ph")
                            for i in range(4):
                                nc.tensor.matmul(
                                    ps[:],
                                    ewh_k[:, l - 1, i, o * 128 : (o + 1) * 128],
                                    h[:, i, :],
                                    start=(i == 0),
                                    stop=(i == 3),
                                )
                            relu_store(
                                hn[:, o, :],
                                ps[:],
                                ebias[:, k, l, o : o + 1],
                                accum_out=(rsum[:, o : o + 1] if l == 2 else None),
                            )
                        h = hn
                    nc.vector.tensor_copy(rr_all[:, :, s], rsum[:])
                # mu/lv heads for both sets at once (N=2); the 1/512 mean
                # factor is folded into the bias-add below
                ps_mu = ps_s.tile([128, 2], f32, tag="pss")
                for i in range(4):
                    nc.tensor.matmul(
                        ps_mu[:], ewml_k[:, i, 0:L], rr_all[:, i, :],
                        start=(i == 0), stop=(i == 3),
                    )
                ps_lv = ps_s.tile([128, 2], f32, tag="pss")
                for i in range(4):
                    nc.tensor.matmul(
                        ps_lv[:], ewml_k[:, i, L : 2 * L], rr_all[:, i, :],
                        start=(i == 0), stop=(i == 3),
                    )
                for s in range(2):
                    nc.vector.scalar_tensor_tensor(
                        out=enc_sb[:, s, 0, k : k + 1],
                        in0=ps_mu[:, s : s + 1],
                        scalar=1.0 / 512.0,
                        in1=emlb[:, k, 0:1],
                        op0=ALU.mult,
                        op1=ALU.add,
                    )
                    nc.vector.scalar_tensor_tensor(
                        out=enc_sb[:, s, 1, k : k + 1],
                        in0=ps_lv[:, s : s + 1],
                        scalar=1.0 / 512.0,
                        in1=emlb[:, k, 1:2],
                        op0=ALU.mult,
                        op1=ALU.add,
                    )
                ze = small.tile([128, 1], f32, tag="ze")
                nc.scalar.activation(
                    out=ze[:],
                    in_=enc_sb[:, 1, 1, k : k + 1],
                    func=AF.Exp,
                    scale=0.5,
                )
                zm = small.tile([128, 1], f32, tag="zm")
                nc.vector.tensor_mul(zm[:], ze[:], epsT[:, k : k + 1])
                nc.vector.tensor_add(
                    zT[:, k : k + 1], zm[:], enc_sb[:, 1, 0, k : k + 1]
                )
            nc.sync.dma_start(d_enc_out[:], enc_sb[:])

            # phase-D weights: loaded after the encoder weights so the
            # startup DMA queue serves the encoder first (these are not
            # needed until the encoder phase is done)
            dw0z = consts.tile([128, DH], f32r)
            nc.sync.dma_start(dw0z[:], d_dw0z[:])
            dwh = consts.tile([128, NDH, 4, DH], f32r)
            nc.sync.dma_start(dwh[:], d_dwh[:])
            dwo = consts.tile([128, 4, 35], f32r)
            nc.sync.dma_start(dwo[:], d_dwo[:])
            pw0z = consts.tile([128, HG], f32r)
            nc.sync.dma_start(pw0z[:], d_pw0z[:])
            pwh = consts.tile([128, 2, HG], f32r)
            nc.sync.dma_start(pwh[:], d_pwh[:])
            pwo = consts.tile([128, 2, 1], f32r)
            nc.sync.dma_start(pwo[:], d_pwo[:])
            qw0z = consts.tile([128, HG], f32r)
            nc.sync.dma_start(qw0z[:], d_qw0z[:])
            qwh = consts.tile([128, 2, HG], f32r)
            nc.sync.dma_start(qwh[:], d_qwh[:])
            qwo = consts.tile([128, 2, 1], f32r)
            nc.sync.dma_start(qwo[:], d_qwo[:])
            # ---------------- z-dependent biases, all experts at once ----
            # zball slots: 0..3 decoder o-tiles, 4..5 post gate, 6..7 prior
            zball = persist.tile([128, 8, K], f32)
            zb_specs = (
                [(dw0z, o, dbias[:, 0, o : o + 1], o) for o in range(4)]
                + [(pw0z, o, pb[:, 0, o : o + 1], 4 + o) for o in range(2)]
                + [(qw0z, o, qb[:, 0, o : o + 1], 6 + o) for o in range(2)]
            )
            for zw, o, bias_ap, slot in zb_specs:
                ps = ps_s.tile([128, K], f32, tag="pss")
                nc.tensor.matmul(
                    ps[:], zw[:, o * 128 : (o + 1) * 128], zT[:], start=True, stop=True
                )
                nc.vector.tensor_scalar(
                    zball[:, slot, :], ps[:], bias_ap, zcol[:, 0:1], ALU.add, ALU.add
                )

            # ---------------- per-expert gates + decoder ----------------
            def emit_g0(k):
                # gate L0 relus for expert k; emitted one expert ahead so the
                # ACT/DVE engines produce them while the PE runs the previous
                # expert's decoder (kills the PE stall at expert boundaries)
                res = []
                for xp, zslice in (
                    (xpp, zball[:, 4:6, k : k + 1]),
                    (xpq, zball[:, 6:8, k : k + 1]),
                ):
                    per_c = []
                    for c in range(2):
                        g0 = g0pool.tile([128, 2, 512], f32r, tag="g0")
                        for o in range(2):
                            relu_store(
                                g0[:, o, :],
                                xp[:, o, c * 512 : (c + 1) * 512],
                                zslice[:, o, :],
                                sbuf_src=True,
                            )
                        per_c.append(g0)
                    res.append(per_c)
                return res

            def gate_stage1(k, gi):
                # gate hidden layer (PE + relu); g0 was produced during the
                # previous expert's decoder
                wh, gb = ((pwh, pb), (qwh, qb))[gi]
                per_c = []
                for c in range(2):
                    g0 = g0_cur[gi][c]
                    g1 = ghpool.tile([128, 2, 512], f32r, tag="gh")
                    for o in range(2):
                        ps = ps_h.tile([128, 512], f32, tag="ph")
                        for i in range(2):
                            nc.tensor.matmul(
                                ps[:],
                                wh[:, i, o * 128 : (o + 1) * 128],
                                g0[:, i, :],
                                start=(i == 0),
                                stop=(i == 1),
                            )
                        relu_store(g1[:, o, :], ps[:], gb[:, 1, o : o + 1])
                    per_c.append(g1)
                return per_c

            def gate_stage2(k, gi, g1s):
                # logits (weight-stationary, M=1) + softmax-layout scatter
                wo = (pwo, qwo)[gi]
                asb = (asb_p, asb_q)[gi]
                for c in range(2):
                    psl = ps_l.tile([1, 512], f32, tag="psl")
                    for i in range(2):
                        nc.tensor.matmul(
                            psl[:],
                            wo[:, i, 0:1],
                            g1s[c][:, i, :],
                            start=(i == 0),
                            stop=(i == 1),
                        )
                    lgc = outs.tile([1, 512], f32, tag="lg")
                    nc.vector.tensor_copy(lgc[:], psl[:])
                    nc.sync.dma_start(d_lg[gi, k, c * 512 : (c + 1) * 512], lgc[:])
                nc.sync.dma_start(
                    asb[:, :, k],
                    d_lg[gi, k].rearrange("(c p) -> p c", p=128),
                )

            def emit_softmax():
                for asb, dout in ((asb_p, d_ap_out), (asb_q, d_aq_out)):
                    for r in range(8):
                        nm = smx.tile([128, 1], f32, tag="sm")
                        nc.vector.tensor_reduce(
                            out=nm[:], in_=asb[:, r, :], axis=AX.X, op=ALU.max
                        )
                        nc.scalar.mul(nm[:], nm[:], -1.0)
                        e = smx.tile([128, K], f32, tag="se")
                        nc.scalar.activation(
                            out=e[:], in_=asb[:, r, :], func=AF.Exp, bias=nm[:, 0:1]
                        )
                        ssum = smx.tile([128, 1], f32, tag="ss")
                        nc.vector.tensor_reduce(
                            out=ssum[:], in_=e[:], axis=AX.X, op=ALU.add
                        )
                        rec = smx.tile([128, 1], f32, tag="sr")
                        nc.vector.reciprocal_approx_fast(out=rec[:], in_=ssum[:])
                        a = smx.tile([128, K], f32, tag="sa")
                        nc.vector.tensor_scalar_mul(a[:], e[:], rec[:, 0:1])
                        nc.sync.dma_start(dout[r * 128 : (r + 1) * 128, :], a[:])

            # the gate pipeline for expert k is spread through the decoder of
            # expert k, stage by stage, so the PE stream never drains on the
            # gate latency chain (matmul -> relu -> matmul -> copy)
            g0_cur = emit_g0(0)
            g1_cur = [gate_stage1(0, 0), gate_stage1(0, 1)]
            for k in range(K):
                zb_d = zball[:, 0:4, k : k + 1]
                ymo = outs.tile([35, T], f32, tag="ymo")
                for c in range(2):
                    h = acts3.tile([128, 4, 512], f32r, tag="dech")
                    for o in range(4):
                        relu_store(
                            h[:, o, :],
                            xpd[:, o, c * 512 : (c + 1) * 512],
                            zb_d[:, o, :],
                            sbuf_src=True,
                        )
                    for l in range(NDH):
                        hn = acts3.tile([128, 4, 512], f32r, tag="dech")
                        for o in range(4):
                            ps = ps_h.tile([128, 512], f32, tag="ph")
                            for i in range(4):
                                nc.tensor.matmul(
                                    ps[:],
                                    dwh[:, l, i, o * 128 : (o + 1) * 128],
                                    h[:, i, :],
                                    start=(i == 0),
                                    stop=(i == 3),
                                )
                            relu_store(hn[:, o, :], ps[:], dbias[:, l + 1, o : o + 1])
                        h = hn
                        if c == 0 and l == 0:
                            gate_stage2(k, 0, g1_cur[0])
                        elif c == 0 and l == 1:
                            gate_stage2(k, 1, g1_cur[1])
                            if k == K - 1:
                                emit_softmax()
                        elif c == 0 and l == 2 and k + 1 < K:
                            g0_cur = emit_g0(k + 1)
                        elif c == 1 and l == 0 and k + 1 < K:
                            g1_cur[0] = gate_stage1(k + 1, 0)
                        elif c == 1 and l == 1 and k + 1 < K:
                            g1_cur[1] = gate_stage1(k + 1, 1)
                    pso = ps_o.tile([35, 512], f32, tag="po")
                    for i in range(4):
                        nc.tensor.matmul(
                            pso[:],
                            dwo[:, i, :],
                            h[:, i, :],
                            start=(i == 0),
                            stop=(i == 3),
                        )
                    # sigmoid(x+b) = 1/(1+exp(-(x+b)));  dbo holds -b in rows
                    # 0:3 and +b in rows 3:6 (prepared host-side)
                    et = small.tile([35, 512], f32, tag="eo")
                    nc.scalar.activation(
                        out=et[0:3, :],
                        in_=pso[0:3, :],
                        func=AF.Exp,
                        bias=dbo[0:3, 0:1],
                        scale=-1.0,
                    )
                    nc.scalar.activation(
                        out=et[32:35, :],
                        in_=pso[32:35, :],
                        func=AF.Exp,
                        bias=dbo[32:35, 0:1],
                    )
                    nc.vector.tensor_scalar_add(et[:], et[:], 1.0)
                    nc.vector.reciprocal_approx_fast(
                        out=ymo[0:3, c * 512 : (c + 1) * 512], in_=et[0:3, :]
                    )
                    # softplus(x+b) = ln(1 + exp(x+b))
                    nc.scalar.activation(
                        out=ymo[32:35, c * 512 : (c + 1) * 512],
                        in_=et[32:35, :],
                        func=AF.Ln,
                    )
                nc.sync.dma_start(d_dec_out[k, 0:3], ymo[0:3, :])
                nc.sync.dma_start(d_dec_out[k, 3:6], ymo[32:35, :])


    nc.compile()
    return nc


def _prep_shared(inp):
    """Host-side weight layout transforms (same for all cores)."""
    f = np.ascontiguousarray
    eW0, eb0, eWh, ebh = inp["eW0"], inp["eb0"], inp["eWh"], inp["ebh"]
    eWmu, ebmu, eWlv, eblv = inp["eWmu"], inp["ebmu"], inp["eWlv"], inp["eblv"]
    pW0, pb0, pWh, pbh, pWo = inp["pW0"], inp["pb0"], inp["pWh"], inp["pbh"], inp["pWo"]
    qW0, qb0, qWh, qbh, qWo = inp["qW0"], inp["qb0"], inp["qWh"], inp["qbh"], inp["qWo"]
    dW0, db0, dWh, dbh, dWo, dbo = (
        inp["dW0"], inp["db0"], inp["dWh"], inp["dbh"], inp["dWo"], inp["dbo"],
    )

    out = {}
    out["ew0"] = f(eW0)  # [K, 5, H]
    # ewh[k, p, l, i, o] = eWh[l, k, i*128+p, o]
    ewh = eWh.reshape(NH, K, 4, 128, H).transpose(1, 3, 0, 2, 4)
    out["ewh"] = f(ewh)
    # ewml[k, p, i, :128]=eWmu[k, i*128+p, :], [128:]=eWlv
    wmu = eWmu.reshape(K, 4, 128, L).transpose(0, 2, 1, 3)
    wlv = eWlv.reshape(K, 4, 128, L).transpose(0, 2, 1, 3)
    out["ewml"] = f(np.concatenate([wmu, wlv], axis=-1))  # [K,128,4,256]
    # ebias[p, k, l, o]: l=0 -> eb0[k, o*128+p]; l=1,2 -> ebh[l-1, k, o*128+p]
    eb_all = np.stack([eb0, ebh[0], ebh[1]], axis=1)  # [K, 3, H]
    out["ebias"] = f(eb_all.reshape(K, 3, 4, 128).transpose(3, 0, 1, 2))
    out["emlb"] = f(np.stack([ebmu, eblv], axis=-1).transpose(1, 0, 2))  # [128,K,2]

    out["dw0x"] = f(dW0[:2])  # [2, DH]
    out["dw0z"] = f(dW0[2:])  # [128, DH]
    out["dwh"] = f(dWh.reshape(NDH, 4, 128, DH).transpose(2, 0, 1, 3))  # [128,3,4,DH]
    dwo_t = dWo.reshape(4, 128, 6).transpose(1, 0, 2)  # [128,4,6]
    dwo_pad = np.zeros((128, 4, 35), dWo.dtype)
    dwo_pad[:, :, 0:3] = dwo_t[:, :, 0:3]
    dwo_pad[:, :, 32:35] = dwo_t[:, :, 3:6]
    out["dwo"] = f(dwo_pad)
    db_all = np.stack([db0, dbh[0], dbh[1], dbh[2]], axis=0)  # [4, DH]
    out["dbias"] = f(db_all.reshape(4, 4, 128).transpose(2, 0, 1))  # [128,4,4]
    # rows 0:3 hold -bias (sigmoid via exp(-(x+b))), rows 32:35 hold +bias
    dbo_pad = np.zeros((35, 1), dbo.dtype)
    dbo_pad[0:3, 0] = -dbo[:3]
    dbo_pad[32:35, 0] = dbo[3:]
    out["dbo"] = f(dbo_pad)

    out["pw0i"] = f(pW0[:5])
    out["pw0z"] = f(pW0[5:])
    out["pwh"] = f(pWh[0].reshape(2, 128, HG).transpose(1, 0, 2))  # [128,2,HG]
    out["pwo"] = f(pWo.reshape(2, 128, 1).transpose(1, 0, 2))  # [128,2,1]
    pb_all = np.stack([pb0, pbh[0]], axis=0)  # [2, HG]
    out["pb"] = f(pb_all.reshape(2, 2, 128).transpose(2, 0, 1))  # [128,2,2]

    out["qw0i"] = f(qW0[:2])
    out["qw0z"] = f(qW0[2:])
    out["qwh"] = f(qWh[0].reshape(2, 128, HG).transpose(1, 0, 2))
    out["qwo"] = f(qWo.reshape(2, 128, 1).transpose(1, 0, 2))
    qb_all = np.stack([qb0, qbh[0]], axis=0)
    out["qb"] = f(qb_all.reshape(2, 2, 128).transpose(2, 0, 1))
    return {k2: np.asarray(v, np.float32) for k2, v in out.items()}


def kernel(**inputs):
    from concourse.bass_utils import run_bass_kernel_spmd
    import os

    inputs = {k2: np.asarray(v, np.float32) for k2, v in inputs.items()}
    if "nc" not in _CACHE:
        _CACHE["nc"] = _build()
    nc = _CACHE["nc"]

    shared = _prep_shared(inputs)
    x_c, y_c = inputs["x_c"], inputs["y_c"]
    x_t, y_t = inputs["x_t"], inputs["y_t"]
    x_pred, y_pred, eps = inputs["x_pred"], inputs["y_pred"], inputs["eps"]

    in_maps = []
    for b in range(B):
        m = dict(shared)
        memo_c = np.concatenate([x_c[b], y_c[b]], axis=-1).T  # [5, NC]
        memo_t = np.concatenate([x_t[b], y_t[b]], axis=-1).T  # [5, NT]
        m["enc_in"] = np.ascontiguousarray(
            np.stack([memo_c, memo_t], axis=1), np.float32
        )  # [5, 2, 512]
        m["xyT"] = np.ascontiguousarray(
            np.concatenate([x_pred[b], y_pred[b]], axis=-1).T, np.float32
        )  # [5, T]
        m["epsT"] = np.ascontiguousarray(eps[b].T, np.float32)  # [128, K]
        in_maps.append(m)

    trace = bool(int(os.environ.get("BASS_KERNEL_TRACE", "0")))
    if trace:
        try:
            import trnprof

            trnprof.install()
        except Exception:
            trace = False
    res = run_bass_kernel_spmd(nc, in_maps, core_ids=list(range(B)), trace=trace)
    _CACHE["exec_time_ns"] = res.exec_time_ns

    mu_c = np.empty((B, K, L), np.float32)
    lv_c = np.empty((B, K, L), np.float32)
    mu_t = np.empty((B, K, L), np.float32)
    lv_t = np.empty((B, K, L), np.float32)
    y_mean = np.empty((B, T, K, DY), np.float32)
    y_std = np.empty((B, T, K, DY), np.float32)
    alpha_post = np.empty((B, T, K), np.float32)
    alpha_prior = np.empty((B, T, K), np.float32)
    for b in range(B):
        r = res.results[b]
        eo = r["enc_out"]  # [128, 2, 2, K]
        mu_c[b] = eo[:, 0, 0, :].T
        lv_c[b] = eo[:, 0, 1, :].T
        mu_t[b] = eo[:, 1, 0, :].T
        lv_t[b] = eo[:, 1, 1, :].T
        do = r["dec_out"]  # [K, 6, T]
        y_mean[b] = do[:, 0:3, :].transpose(2, 0, 1)
        y_std[b] = do[:, 3:6, :].transpose(2, 0, 1)
        alpha_post[b] = r["alpha_post"]
        alpha_prior[b] = r["alpha_prior"]

    return (mu_c, lv_c, mu_t, lv_t, y_mean, y_std, alpha_post, alpha_prior)
